# revision 9
# baseline (speedup 1.0000x reference)
"""GNN message passing (copy_src + segment_sum + Linear + ReLU) on 8 TRN2 cores.

v2. Structure: dst nodes are bin-packed (host side) into 392 windows =
8 cores x 49 slots, <=128 nodes per window, with per-slot uniform edge-tile
capacities (KA_s, KB_s) shared by all cores so the instruction stream is
SPMD-identical. Each core gathers the src rows of its edges from a bf16
replica of the feature table (split in two halves at node 25000 for the
int16 gather-index range) with 1024-index dma_gather calls rotated over
4 SWDGE queues (descriptor generation parallelizes ~4x across queues;
measured 2.6 ns/idx vs 4.6 at 2 queues). Aggregation on-chip: per 128-edge
tile a one-hot(dst_lane) bf16 matrix is built with DVE is_equal, and PE
matmuls accumulate aggT[f, lane] += msgs[e, f].T @ onehot[e, lane] in PSUM
(fp32). Node update per window: aggT -> bf16, matmul with W^T + K=1 bias
matmul, ACT ReLU, DMA out fp32. Host assembles the full output from the
window node lists. No collectives.

Self-contained: shapes hardcoded for feature[50000,128], src/dst[640000],
W[128,128], b[128].
"""
import dataclasses

import numpy as np
import ml_dtypes

import concourse.bacc as bacc
import concourse.tile as tile
from concourse import mybir
from concourse.bass_utils import run_bass_kernel_spmd

P = 128
N_NODES = 50000
N_EDGES = 640000
VHALF = 25000
NC = 8
W_SLOTS = 49
NBINS = NC * W_SLOTS                 # 392 windows
CALL_TILES = 8                       # 1024 idx per dma_gather (ring capacity)
NQ = 4                               # SWDGE queues
BATCH_SLOTS = 4

F32 = mybir.dt.float32
BF16 = mybir.dt.bfloat16
I16 = mybir.dt.int16
BF = ml_dtypes.bfloat16


def _slot_caps(extra14):
    """Per-slot (KA, KB): slots 0..24 are (7,6), 25..48 are (6,7); the first
    `extra14` slots of each group get +1 on the minor side (K=14)."""
    caps = []
    for s in range(W_SLOTS):
        if s < 25:
            ka, kb = 7, 6
            if s < extra14:
                kb += 1
        else:
            ka, kb = 6, 7
            if s - 25 < extra14:
                ka += 1
        caps.append((ka, kb))
    return caps


def _pack_nodes(da, db, caps):
    """Assign all nodes to NBINS bins with per-bin caps (128 nodes,
    KA*128 A-edges, KB*128 B-edges). Vectorized greedy, high-degree first.
    Returns bins: list of node-id arrays, or None if packing fails."""
    n = da + db
    order = np.argsort(-(n.astype(np.int64) * 4 + (da > db)))
    a_cap = np.array([caps[b % W_SLOTS][0] * P for b in range(NBINS)],
                     dtype=np.int64)
    b_cap = np.array([caps[b % W_SLOTS][1] * P for b in range(NBINS)],
                     dtype=np.int64)
    a_left = a_cap.copy()
    b_left = b_cap.copy()
    n_left = np.full(NBINS, P, dtype=np.int64)
    assign = np.empty(N_NODES, dtype=np.int64)
    for node in order:
        ai, bi = da[node], db[node]
        feas = (n_left > 0) & (a_left >= ai) & (b_left >= bi)
        if not feas.any():
            return None
        # prefer the bin whose remaining a:b slack best matches this node's
        # skew, with a tie-break toward emptier bins
        slack = np.minimum(a_left - ai, b_left - bi).astype(np.float64)
        score = slack + 0.25 * (a_left + b_left - ai - bi)
        score[~feas] = -1e18
        bsel = int(np.argmax(score))
        assign[node] = bsel
        a_left[bsel] -= ai
        b_left[bsel] -= bi
        n_left[bsel] -= 1
    bins = [np.where(assign == b)[0] for b in range(NBINS)]
    return bins


def _make_plan(src, dst):
    src = np.asarray(src, dtype=np.int64)
    dst = np.asarray(dst, dtype=np.int64)
    isa = src < VHALF
    da = np.bincount(dst[isa], minlength=N_NODES)
    db = np.bincount(dst[~isa], minlength=N_NODES)

    for extra14 in (0, 2, 4, 8, 16, 25):
        caps = _slot_caps(extra14)
        bins = _pack_nodes(da, db, caps)
        if bins is not None:
            break
    else:
        raise RuntimeError("node packing failed")

    KA = np.array([c[0] for c in caps], dtype=np.int64)
    KB = np.array([c[1] for c in caps], dtype=np.int64)
    TA = int(KA.sum())
    TB = int(KB.sum())
    T_tot = TA + TB
    a_base = np.concatenate([[0], np.cumsum(KA)])
    b_base = np.concatenate([[0], np.cumsum(KB)])

    # per-edge bin and lane-in-bin
    node_bin = np.empty(N_NODES, dtype=np.int64)
    node_lane = np.empty(N_NODES, dtype=np.int64)
    for bid, nodes in enumerate(bins):
        node_bin[nodes] = bid
        node_lane[nodes] = np.arange(len(nodes))

    ebin = node_bin[dst]
    # order edges by (bin, half, src) for locality and contiguous fill
    order = np.lexsort((src, ~isa, ebin))
    e_bin = ebin[order]
    e_half = (~isa)[order]            # 0 = A, 1 = B
    e_src = src[order]
    e_lane = node_lane[dst[order]]

    # batches of BATCH_SLOTS slots; per batch, A-calls and B-calls chunk the
    # batch's contiguous A/B tile regions into <=CALL_TILES-tile calls.
    # dstloc columns are stored in BATCH order (batch's A tiles then B tiles)
    # so the one-hot is built with ONE tensor_tensor per batch.
    batches = []
    s = 0
    t_base = 0
    dstcol_a = np.zeros(W_SLOTS, dtype=np.int64)   # dstloc col of slot's tile 0 (A)
    dstcol_b = np.zeros(W_SLOTS, dtype=np.int64)
    while s < W_SLOTS:
        s1 = min(s + BATCH_SLOTS, W_SLOTS)
        slots = list(range(s, s1))
        ka_b = int(KA[s:s1].sum())
        kb_b = int(KB[s:s1].sum())
        wins = []
        pa, pb = 0, 0
        for w in slots:
            dstcol_a[w] = t_base + pa
            dstcol_b[w] = t_base + ka_b + pb
            cols = list(range(pa, pa + int(KA[w]))) + \
                   list(range(ka_b + pb, ka_b + pb + int(KB[w])))
            wins.append((w, cols))
            pa += int(KA[w])
            pb += int(KB[w])
        batches.append(dict(ka=ka_b, kb=kb_b, t_base=t_base,
                            a0=int(a_base[s]), b0=int(b_base[s]), wins=wins))
        t_base += ka_b + kb_b
        s = s1

    dstloc = np.full((NC, P, T_tot), -1.0, dtype=np.float64)
    idxA = np.zeros((NC, TA * P), dtype=np.int16)
    idxB = np.zeros((NC, TB * P), dtype=np.int16)

    # bincount per (bin, half)
    key = e_bin * 2 + e_half
    cnt = np.bincount(key, minlength=NBINS * 2)
    starts = np.concatenate([[0], np.cumsum(cnt)])
    for bid in range(NBINS):
        c, s = divmod(bid, W_SLOTS)
        for half in (0, 1):
            k = bid * 2 + half
            e0, e1 = starts[k], starts[k + 1]
            m = e1 - e0
            if m == 0:
                continue
            j = np.arange(m)
            if half == 0:
                assert m <= KA[s] * P, (bid, m, KA[s] * P)
                dstloc[c, j % P, dstcol_a[s] + j // P] = e_lane[e0:e1]
                idxA[c, a_base[s] * P + j] = e_src[e0:e1].astype(np.int16)
            else:
                assert m <= KB[s] * P, (bid, m, KB[s] * P)
                dstloc[c, j % P, dstcol_b[s] + j // P] = e_lane[e0:e1]
                idxB[c, b_base[s] * P + j] = (e_src[e0:e1] - VHALF).astype(np.int16)

    return dict(bins=bins, caps=caps, KA=KA, KB=KB, TA=TA, TB=TB,
                T_tot=T_tot, batches=batches, dstloc=dstloc,
                idxA=idxA, idxB=idxB)


def _wrap16(idx_flat):
    n = idx_flat.shape[0]
    arr = np.empty((16, n // 16), dtype=np.int16)
    j = np.arange(n)
    arr[j % 16, j // 16] = idx_flat
    return np.tile(arr, (8, 1))


def _build_nc(plan):
    TA, TB, T_tot = plan["TA"], plan["TB"], plan["T_tot"]
    # const fp32 column layout
    c_dst = 0                               # dstloc bf16 [P, T_tot]
    dst_f = (T_tot + 1) // 2
    c_ia = c_dst + dst_f                    # idxA int16 [P, TA*8]
    c_ib = c_ia + TA * 4                    # idxB int16 [P, TB*8]
    c_iota = c_ib + TB * 4                  # iota bf16 [P, P]
    c_wt = c_iota + P // 2                  # W^T bf16 [P, P]
    c_brow = c_wt + P // 2                  # bias row bf16 [1, P]
    c_tot = c_brow + P // 2
    plan["c_layout"] = (c_dst, c_ia, c_ib, c_iota, c_wt, c_brow, c_tot)

    nc = bacc.Bacc("TRN2", num_swdge_queues=NQ)
    featA = nc.declare_dram_parameter("featA", [VHALF, P], BF16, isOutput=False)
    featB = nc.declare_dram_parameter("featB", [N_NODES - VHALF, P], BF16,
                                      isOutput=False)
    consts = nc.declare_dram_parameter("consts", [P, c_tot], F32, isOutput=False)
    out = nc.declare_dram_parameter("out", [W_SLOTS * P, P], F32, isOutput=True)

    with tile.TileContext(nc) as tc:
        with (
            tc.tile_pool(name="const", bufs=1) as const_pool,
            tc.tile_pool(name="msgs", bufs=3) as msgs_pool,
            tc.tile_pool(name="oneh", bufs=3) as oneh_pool,
            tc.tile_pool(name="outp", bufs=3) as out_pool,
            tc.tile_pool(name="psum", bufs=4, space="PSUM") as psum_pool,
        ):
            cs = const_pool.tile([P, c_tot], F32)
            nc.sync.dma_start(out=cs[:], in_=consts[:])
            dst_sb = cs[:, c_dst:c_ia].bitcast(BF16)
            idxA_sb = cs[:, c_ia:c_ib].bitcast(I16)
            idxB_sb = cs[:, c_ib:c_iota].bitcast(I16)
            iota_sb = cs[:, c_iota:c_wt].bitcast(BF16)
            wt_sb = cs[:, c_wt:c_brow].bitcast(BF16)
            brow_sb = cs[0:1, c_brow:c_tot].bitcast(BF16)
            ones_sb = const_pool.tile([1, P], BF16)
            nc.vector.memset(ones_sb[:], 1.0)

            gq = [0]

            def gather_calls(msgs, coff, k_tiles, feat, idx_sb, flat0):
                # chunk [0, k_tiles) into CALL_TILES-tile calls (full tiles
                # first; ring/packet capacity caps a call at 1024 indices)
                sizes = [min(CALL_TILES, k_tiles - o)
                         for o in range(0, k_tiles, CALL_TILES)]
                off = 0
                for nk in sizes:
                    nc.gpsimd.dma_gather(
                        out_ap=msgs[:, coff + off:coff + off + nk, :],
                        in_ap=feat[:],
                        idxs_ap=idx_sb[:, (flat0 + off) * 8:(flat0 + off + nk) * 8],
                        num_idxs=nk * P,
                        num_idxs_reg=nk * P,
                        elem_size=P,
                        queue_num=gq[0] % NQ,
                    )
                    gq[0] += 1
                    off += nk

            for bt in plan["batches"]:
                ka_b, kb_b = bt["ka"], bt["kb"]
                k_b = ka_b + kb_b
                msgs = msgs_pool.tile([P, k_b, P], BF16, tag="msgs")
                gather_calls(msgs, 0, ka_b, featA, idxA_sb, bt["a0"])
                gather_calls(msgs, ka_b, kb_b, featB, idxB_sb, bt["b0"])

                onehot = oneh_pool.tile([P, k_b, P], BF16, tag="onehot")
                # dstloc columns are stored in batch order: one build per batch
                nc.vector.tensor_tensor(
                    out=onehot[:],
                    in0=dst_sb[:, bt["t_base"]:bt["t_base"] + k_b]
                        .to_broadcast([P, k_b, P]),
                    in1=dataclasses.replace(
                        iota_sb, ap=[iota_sb.ap[0], [0, k_b], iota_sb.ap[1]]),
                    op=mybir.AluOpType.is_equal,
                )
                for w, cols in bt["wins"]:
                    aggT_ps = psum_pool.tile([P, P], F32, tag="aggT")
                    for i, ccol in enumerate(cols):
                        nc.tensor.matmul(
                            out=aggT_ps[:],
                            lhsT=msgs[:, ccol, :],
                            rhs=onehot[:, ccol, :],
                            start=(i == 0),
                            stop=(i == len(cols) - 1),
                        )
                    aggT_sb = out_pool.tile([P, P], BF16, tag="aggT_sb")
                    nc.vector.tensor_copy(out=aggT_sb[:], in_=aggT_ps[:])
                    out2_ps = psum_pool.tile([P, P], F32, tag="out2")
                    nc.tensor.matmul(out=out2_ps[:], lhsT=aggT_sb[:], rhs=wt_sb,
                                     start=True, stop=False)
                    nc.tensor.matmul(out=out2_ps[:], lhsT=ones_sb[:], rhs=brow_sb,
                                     start=False, stop=True)
                    out_sb = out_pool.tile([P, P], F32, tag="out_sb")
                    nc.scalar.activation(out=out_sb[:], in_=out2_ps[:],
                                         func=mybir.ActivationFunctionType.Relu)
                    nc.sync.dma_start(out=out[w * P:(w + 1) * P, :], in_=out_sb[:])
    nc.finalize()
    return nc


_CACHE = {}


def _prepare(feature, src, dst, W, b):
    feature = np.asarray(feature, dtype=np.float32)
    W = np.asarray(W, dtype=np.float32)
    b = np.asarray(b, dtype=np.float32)
    key = (hash(np.asarray(src).tobytes()), hash(np.asarray(dst).tobytes()))
    if key not in _CACHE:
        plan = _make_plan(src, dst)
        nc = _build_nc(plan)
        _CACHE.clear()
        _CACHE[key] = (plan, nc)
    plan, nc = _CACHE[key]
    c_dst, c_ia, c_ib, c_iota, c_wt, c_brow, c_tot = plan["c_layout"]
    TA, TB, T_tot = plan["TA"], plan["TB"], plan["T_tot"]
    iota = np.arange(P, dtype=BF)
    featA = np.ascontiguousarray(feature[:VHALF].astype(BF))
    featB = np.ascontiguousarray(feature[VHALF:].astype(BF))
    in_maps = []
    for c in range(NC):
        consts = np.zeros((P, c_tot), dtype=np.float32)

        def put_bf16(col0, arr2d):
            # arr2d [rows, cols] bf16 -> fp32 columns starting at col0
            a = np.asarray(arr2d, dtype=BF)
            rows, cols = a.shape
            pad = (-cols) % 2
            if pad:
                a = np.concatenate([a, np.zeros((rows, pad), BF)], axis=1)
            a = np.ascontiguousarray(a)
            consts[:rows, col0:col0 + a.shape[1] // 2] = a.view(np.float32)

        put_bf16(c_dst, plan["dstloc"][c])
        consts[:, c_ia:c_ib] = _wrap16(plan["idxA"][c]).view(np.float32)
        consts[:, c_ib:c_iota] = _wrap16(plan["idxB"][c]).view(np.float32)
        put_bf16(c_iota, np.broadcast_to(iota[None, :], (P, P)))
        put_bf16(c_wt, W.T.astype(BF))
        put_bf16(c_brow, b[None, :].astype(BF))
        in_maps.append({"featA": featA, "featB": featB, "consts": consts})
    return plan, nc, in_maps


def _assemble(plan, results):
    out_full = np.zeros((N_NODES, P), dtype=np.float32)
    for c in range(NC):
        oc = results[c]["out"]
        for s in range(W_SLOTS):
            nodes = plan["bins"][c * W_SLOTS + s]
            if len(nodes):
                out_full[nodes] = oc[s * P:s * P + len(nodes)]
    return out_full


def kernel(feature, src, dst, W, b):
    plan, nc, in_maps = _prepare(feature, src, dst, W, b)
    res = run_bass_kernel_spmd(nc, in_maps, list(range(NC)))
    return _assemble(plan, res.results)


# revision 13
# speedup vs baseline: 1.3688x; 1.3688x over previous
"""GNN message passing (copy_src + segment_sum + Linear + ReLU) on 8 TRN2 cores.

v2. Structure: dst nodes are bin-packed (host side) into 392 windows =
8 cores x 49 slots, <=128 nodes per window, with per-slot uniform edge-tile
capacities (KA_s, KB_s) shared by all cores so the instruction stream is
SPMD-identical. Each core gathers the src rows of its edges from a bf16
replica of the feature table (split in two halves at node 25000 for the
int16 gather-index range) with 1024-index dma_gather calls rotated over
4 SWDGE queues (descriptor generation parallelizes ~4x across queues;
measured 2.6 ns/idx vs 4.6 at 2 queues). Aggregation on-chip: per 128-edge
tile a one-hot(dst_lane) bf16 matrix is built with DVE is_equal, and PE
matmuls accumulate aggT[f, lane] += msgs[e, f].T @ onehot[e, lane] in PSUM
(fp32). Node update per window: aggT -> bf16, matmul with W^T + K=1 bias
matmul, ACT ReLU, DMA out fp32. Host assembles the full output from the
window node lists. No collectives.

Self-contained: shapes hardcoded for feature[50000,128], src/dst[640000],
W[128,128], b[128].
"""
import dataclasses

import numpy as np
import ml_dtypes

import concourse.bacc as bacc
import concourse.tile as tile
from concourse import mybir
from concourse.bass_utils import run_bass_kernel_spmd

P = 128
N_NODES = 50000
N_EDGES = 640000
VHALF = 25000
NC = 8
W_SLOTS = 49
NBINS = NC * W_SLOTS                 # 392 windows
CALL_TILES = 8                       # 1024 idx per dma_gather (ring capacity)
NQ = 4                               # SWDGE queues
BATCH_SLOTS = 4

F32 = mybir.dt.float32
BF16 = mybir.dt.bfloat16
I16 = mybir.dt.int16
BF = ml_dtypes.bfloat16


def _slot_caps(extra14):
    """Per-slot (KA, KB): slots 0..24 are (7,6), 25..48 are (6,7); the first
    `extra14` slots of each group get +1 on the minor side (K=14)."""
    caps = []
    for s in range(W_SLOTS):
        if s < 25:
            ka, kb = 7, 6
            if s < extra14:
                kb += 1
        else:
            ka, kb = 6, 7
            if s - 25 < extra14:
                ka += 1
        caps.append((ka, kb))
    return caps


def _pack_nodes(da, db, caps):
    """Assign all nodes to NBINS bins with per-bin caps (128 nodes,
    KA*128 A-edges, KB*128 B-edges). Vectorized greedy, high-degree first.
    Returns bins: list of node-id arrays, or None if packing fails."""
    n = da + db
    order = np.argsort(-(n.astype(np.int64) * 4 + (da > db)))
    a_cap = np.array([caps[b % W_SLOTS][0] * P for b in range(NBINS)],
                     dtype=np.int64)
    b_cap = np.array([caps[b % W_SLOTS][1] * P for b in range(NBINS)],
                     dtype=np.int64)
    a_left = a_cap.copy()
    b_left = b_cap.copy()
    n_left = np.full(NBINS, P, dtype=np.int64)
    assign = np.empty(N_NODES, dtype=np.int64)
    for node in order:
        ai, bi = da[node], db[node]
        feas = (n_left > 0) & (a_left >= ai) & (b_left >= bi)
        if not feas.any():
            return None
        # prefer the bin whose remaining a:b slack best matches this node's
        # skew, with a tie-break toward emptier bins
        slack = np.minimum(a_left - ai, b_left - bi).astype(np.float64)
        score = slack + 0.25 * (a_left + b_left - ai - bi)
        score[~feas] = -1e18
        bsel = int(np.argmax(score))
        assign[node] = bsel
        a_left[bsel] -= ai
        b_left[bsel] -= bi
        n_left[bsel] -= 1
    bins = [np.where(assign == b)[0] for b in range(NBINS)]
    return bins


def _make_plan(src, dst):
    src = np.asarray(src, dtype=np.int64)
    dst = np.asarray(dst, dtype=np.int64)
    isa = src < VHALF
    da = np.bincount(dst[isa], minlength=N_NODES)
    db = np.bincount(dst[~isa], minlength=N_NODES)

    for extra14 in (0, 2, 4, 8, 16, 25):
        caps = _slot_caps(extra14)
        bins = _pack_nodes(da, db, caps)
        if bins is not None:
            break
    else:
        raise RuntimeError("node packing failed")

    KA = np.array([c[0] for c in caps], dtype=np.int64)
    KB = np.array([c[1] for c in caps], dtype=np.int64)
    TA = int(KA.sum())
    TB = int(KB.sum())
    T_tot = TA + TB
    a_base = np.concatenate([[0], np.cumsum(KA)])
    b_base = np.concatenate([[0], np.cumsum(KB)])

    # per-edge bin and lane-in-bin
    node_bin = np.empty(N_NODES, dtype=np.int64)
    node_lane = np.empty(N_NODES, dtype=np.int64)
    for bid, nodes in enumerate(bins):
        node_bin[nodes] = bid
        node_lane[nodes] = np.arange(len(nodes))

    ebin = node_bin[dst]
    # order edges by (bin, half, src) for locality and contiguous fill
    order = np.lexsort((src, ~isa, ebin))
    e_bin = ebin[order]
    e_half = (~isa)[order]            # 0 = A, 1 = B
    e_src = src[order]
    e_lane = node_lane[dst[order]]

    # batches of BATCH_SLOTS slots; per batch, A-calls and B-calls chunk the
    # batch's contiguous A/B tile regions into <=CALL_TILES-tile calls.
    # dstloc columns are stored in BATCH order (batch's A tiles then B tiles)
    # so the one-hot is built with ONE tensor_tensor per batch.
    batches = []
    s = 0
    t_base = 0
    dstcol_a = np.zeros(W_SLOTS, dtype=np.int64)   # dstloc col of slot's tile 0 (A)
    dstcol_b = np.zeros(W_SLOTS, dtype=np.int64)
    while s < W_SLOTS:
        s1 = min(s + BATCH_SLOTS, W_SLOTS)
        slots = list(range(s, s1))
        ka_b = int(KA[s:s1].sum())
        kb_b = int(KB[s:s1].sum())
        wins = []
        pa, pb = 0, 0
        for w in slots:
            dstcol_a[w] = t_base + pa
            dstcol_b[w] = t_base + ka_b + pb
            cols = list(range(pa, pa + int(KA[w]))) + \
                   list(range(ka_b + pb, ka_b + pb + int(KB[w])))
            wins.append((w, cols))
            pa += int(KA[w])
            pb += int(KB[w])
        batches.append(dict(ka=ka_b, kb=kb_b, t_base=t_base,
                            a0=int(a_base[s]), b0=int(b_base[s]), wins=wins))
        t_base += ka_b + kb_b
        s = s1

    dstloc = np.full((NC, P, T_tot), -1.0, dtype=np.float64)
    idxA = np.zeros((NC, TA * P), dtype=np.int16)
    idxB = np.zeros((NC, TB * P), dtype=np.int16)

    # bincount per (bin, half)
    key = e_bin * 2 + e_half
    cnt = np.bincount(key, minlength=NBINS * 2)
    starts = np.concatenate([[0], np.cumsum(cnt)])
    for bid in range(NBINS):
        c, s = divmod(bid, W_SLOTS)
        for half in (0, 1):
            k = bid * 2 + half
            e0, e1 = starts[k], starts[k + 1]
            m = e1 - e0
            if m == 0:
                continue
            j = np.arange(m)
            if half == 0:
                assert m <= KA[s] * P, (bid, m, KA[s] * P)
                dstloc[c, j % P, dstcol_a[s] + j // P] = e_lane[e0:e1]
                idxA[c, a_base[s] * P + j] = e_src[e0:e1].astype(np.int16)
            else:
                assert m <= KB[s] * P, (bid, m, KB[s] * P)
                dstloc[c, j % P, dstcol_b[s] + j // P] = e_lane[e0:e1]
                idxB[c, b_base[s] * P + j] = (e_src[e0:e1] - VHALF).astype(np.int16)

    return dict(bins=bins, caps=caps, KA=KA, KB=KB, TA=TA, TB=TB,
                T_tot=T_tot, batches=batches, dstloc=dstloc,
                idxA=idxA, idxB=idxB)


def _wrap16(idx_flat):
    n = idx_flat.shape[0]
    arr = np.empty((16, n // 16), dtype=np.int16)
    j = np.arange(n)
    arr[j % 16, j // 16] = idx_flat
    return np.tile(arr, (8, 1))


def _build_nc(plan):
    TA, TB, T_tot = plan["TA"], plan["TB"], plan["T_tot"]
    # const fp32 column layout
    c_ia = 0                                # idxA int16 [P, TA*8]
    c_ib = c_ia + TA * 4                    # idxB int16 [P, TB*8]
    c_wt = c_ib + TB * 4                    # W^T bf16 [P, P]
    c_brow = c_wt + P // 2                  # bias row bf16 [1, P]
    c_tot = c_brow + P // 2
    plan["c_layout"] = (c_ia, c_ib, c_wt, c_brow, c_tot)

    nc = bacc.Bacc("TRN2", num_swdge_queues=NQ)
    featA = nc.declare_dram_parameter("featA", [VHALF, P], BF16, isOutput=False)
    featB = nc.declare_dram_parameter("featB", [N_NODES - VHALF, P], BF16,
                                      isOutput=False)
    oneh_d = nc.declare_dram_parameter("oneh", [P, T_tot, P], BF16,
                                       isOutput=False)
    consts = nc.declare_dram_parameter("consts", [P, c_tot], F32, isOutput=False)
    out = nc.declare_dram_parameter("out", [W_SLOTS * P, P], F32, isOutput=True)

    with tile.TileContext(nc) as tc:
        with (
            tc.tile_pool(name="const", bufs=1) as const_pool,
            tc.tile_pool(name="msgs", bufs=2) as msgs_pool,
            tc.tile_pool(name="oneh", bufs=2) as oneh_pool,
            tc.tile_pool(name="outp", bufs=3) as out_pool,
            tc.tile_pool(name="psum", bufs=4, space="PSUM") as psum_pool,
        ):
            cs = const_pool.tile([P, c_tot], F32)
            nc.sync.dma_start(out=cs[:], in_=consts[:])
            idxA_sb = cs[:, c_ia:c_ib].bitcast(I16)
            idxB_sb = cs[:, c_ib:c_wt].bitcast(I16)
            wt_sb = cs[:, c_wt:c_brow].bitcast(BF16)
            brow_sb = cs[0:1, c_brow:c_tot].bitcast(BF16)
            ones_sb = const_pool.tile([1, P], BF16)
            nc.vector.memset(ones_sb[:], 1.0)

            gq = [0]

            def gather_calls(msgs, coff, k_tiles, feat, idx_sb, flat0):
                # chunk [0, k_tiles) into CALL_TILES-tile calls (full tiles
                # first; ring/packet capacity caps a call at 1024 indices)
                sizes = [min(CALL_TILES, k_tiles - o)
                         for o in range(0, k_tiles, CALL_TILES)]
                off = 0
                for nk in sizes:
                    nc.gpsimd.dma_gather(
                        out_ap=msgs[:, coff + off:coff + off + nk, :],
                        in_ap=feat[:],
                        idxs_ap=idx_sb[:, (flat0 + off) * 8:(flat0 + off + nk) * 8],
                        num_idxs=nk * P,
                        num_idxs_reg=nk * P,
                        elem_size=P,
                        queue_num=gq[0] % NQ,
                    )
                    gq[0] += 1
                    off += nk

            for bt in plan["batches"]:
                ka_b, kb_b = bt["ka"], bt["kb"]
                k_b = ka_b + kb_b
                msgs = msgs_pool.tile([P, k_b, P], BF16, tag="msgs")
                gather_calls(msgs, 0, ka_b, featA, idxA_sb, bt["a0"])
                gather_calls(msgs, ka_b, kb_b, featB, idxB_sb, bt["b0"])

                onehot = oneh_pool.tile([P, k_b, P], BF16, tag="onehot")
                # host-precomputed one-hot, streamed via HWDGE (keeps DVE off
                # the SBUF port GpSimd's descriptor rings live on)
                nc.sync.dma_start(
                    out=onehot[:],
                    in_=oneh_d[:, bt["t_base"]:bt["t_base"] + k_b, :])
                for w, cols in bt["wins"]:
                    aggT_ps = psum_pool.tile([P, P], F32, tag="aggT")
                    for i, ccol in enumerate(cols):
                        nc.tensor.matmul(
                            out=aggT_ps[:],
                            lhsT=msgs[:, ccol, :],
                            rhs=onehot[:, ccol, :],
                            start=(i == 0),
                            stop=(i == len(cols) - 1),
                        )
                    aggT_sb = out_pool.tile([P, P], BF16, tag="aggT_sb")
                    nc.scalar.activation(out=aggT_sb[:], in_=aggT_ps[:],
                                         func=mybir.ActivationFunctionType.Copy)
                    out2_ps = psum_pool.tile([P, P], F32, tag="out2")
                    nc.tensor.matmul(out=out2_ps[:], lhsT=aggT_sb[:], rhs=wt_sb,
                                     start=True, stop=False)
                    nc.tensor.matmul(out=out2_ps[:], lhsT=ones_sb[:], rhs=brow_sb,
                                     start=False, stop=True)
                    out_sb = out_pool.tile([P, P], F32, tag="out_sb")
                    nc.scalar.activation(out=out_sb[:], in_=out2_ps[:],
                                         func=mybir.ActivationFunctionType.Relu)
                    nc.sync.dma_start(out=out[w * P:(w + 1) * P, :], in_=out_sb[:])
    nc.finalize()
    return nc


_CACHE = {}


def _prepare(feature, src, dst, W, b):
    feature = np.asarray(feature, dtype=np.float32)
    W = np.asarray(W, dtype=np.float32)
    b = np.asarray(b, dtype=np.float32)
    key = (hash(np.asarray(src).tobytes()), hash(np.asarray(dst).tobytes()))
    if key not in _CACHE:
        plan = _make_plan(src, dst)
        nc = _build_nc(plan)
        _CACHE.clear()
        _CACHE[key] = (plan, nc)
    plan, nc = _CACHE[key]
    c_ia, c_ib, c_wt, c_brow, c_tot = plan["c_layout"]
    TA, TB, T_tot = plan["TA"], plan["TB"], plan["T_tot"]
    featA = np.ascontiguousarray(feature[:VHALF].astype(BF))
    featB = np.ascontiguousarray(feature[VHALF:].astype(BF))
    lanes = np.arange(P, dtype=np.float64)
    in_maps = []
    for c in range(NC):
        consts = np.zeros((P, c_tot), dtype=np.float32)

        def put_bf16(col0, arr2d):
            # arr2d [rows, cols] bf16 -> fp32 columns starting at col0
            a = np.asarray(arr2d, dtype=BF)
            rows, cols = a.shape
            pad = (-cols) % 2
            if pad:
                a = np.concatenate([a, np.zeros((rows, pad), BF)], axis=1)
            a = np.ascontiguousarray(a)
            consts[:rows, col0:col0 + a.shape[1] // 2] = a.view(np.float32)

        consts[:, c_ia:c_ib] = _wrap16(plan["idxA"][c]).view(np.float32)
        consts[:, c_ib:c_wt] = _wrap16(plan["idxB"][c]).view(np.float32)
        put_bf16(c_wt, W.T.astype(BF))
        put_bf16(c_brow, b[None, :].astype(BF))
        oneh = (plan["dstloc"][c][:, :, None] == lanes[None, None, :]).astype(BF)
        in_maps.append({"featA": featA, "featB": featB, "consts": consts,
                        "oneh": np.ascontiguousarray(oneh)})
    return plan, nc, in_maps


def _assemble(plan, results):
    out_full = np.zeros((N_NODES, P), dtype=np.float32)
    for c in range(NC):
        oc = results[c]["out"]
        for s in range(W_SLOTS):
            nodes = plan["bins"][c * W_SLOTS + s]
            if len(nodes):
                out_full[nodes] = oc[s * P:s * P + len(nodes)]
    return out_full


def kernel(feature, src, dst, W, b):
    plan, nc, in_maps = _prepare(feature, src, dst, W, b)
    res = run_bass_kernel_spmd(nc, in_maps, list(range(NC)))
    return _assemble(plan, res.results)


# revision 15
# speedup vs baseline: 1.4032x; 1.0251x over previous
"""GNN message passing (copy_src + segment_sum + Linear + ReLU) on 8 TRN2 cores.

v2. Structure: dst nodes are bin-packed (host side) into 392 windows =
8 cores x 49 slots, <=128 nodes per window, with per-slot uniform edge-tile
capacities (KA_s, KB_s) shared by all cores so the instruction stream is
SPMD-identical. Each core gathers the src rows of its edges from a bf16
replica of the feature table (split in two halves at node 25000 for the
int16 gather-index range) with 1024-index dma_gather calls rotated over
4 SWDGE queues (descriptor generation parallelizes ~4x across queues;
measured 2.6 ns/idx vs 4.6 at 2 queues). Aggregation on-chip: per 128-edge
tile a one-hot(dst_lane) bf16 matrix is built with DVE is_equal, and PE
matmuls accumulate aggT[f, lane] += msgs[e, f].T @ onehot[e, lane] in PSUM
(fp32). Node update per window: aggT -> bf16, matmul with W^T + K=1 bias
matmul, ACT ReLU, DMA out fp32. Host assembles the full output from the
window node lists. No collectives.

Self-contained: shapes hardcoded for feature[50000,128], src/dst[640000],
W[128,128], b[128].
"""
import dataclasses

import numpy as np
import ml_dtypes

import concourse.bacc as bacc
import concourse.tile as tile
from concourse import mybir
from concourse.bass_utils import run_bass_kernel_spmd

P = 128
N_NODES = 50000
N_EDGES = 640000
VHALF = 25000
NC = 8
W_SLOTS = 49
NBINS = NC * W_SLOTS                 # 392 windows
CALL_TILES = 8                       # 1024 idx per dma_gather (ring capacity)
NQ = 4                               # SWDGE queues
BATCH_SLOTS = 8

F32 = mybir.dt.float32
BF16 = mybir.dt.bfloat16
I16 = mybir.dt.int16
BF = ml_dtypes.bfloat16


def _slot_caps(extra14):
    """Per-slot (KA, KB): slots 0..24 are (7,6), 25..48 are (6,7); the first
    `extra14` slots of each group get +1 on the minor side (K=14)."""
    caps = []
    for s in range(W_SLOTS):
        if s < 25:
            ka, kb = 7, 6
            if s < extra14:
                kb += 1
        else:
            ka, kb = 6, 7
            if s - 25 < extra14:
                ka += 1
        caps.append((ka, kb))
    return caps


def _pack_nodes(da, db, caps):
    """Assign all nodes to NBINS bins with per-bin caps (128 nodes,
    KA*128 A-edges, KB*128 B-edges). Vectorized greedy, high-degree first.
    Returns bins: list of node-id arrays, or None if packing fails."""
    n = da + db
    order = np.argsort(-(n.astype(np.int64) * 4 + (da > db)))
    a_cap = np.array([caps[b % W_SLOTS][0] * P for b in range(NBINS)],
                     dtype=np.int64)
    b_cap = np.array([caps[b % W_SLOTS][1] * P for b in range(NBINS)],
                     dtype=np.int64)
    a_left = a_cap.copy()
    b_left = b_cap.copy()
    n_left = np.full(NBINS, P, dtype=np.int64)
    assign = np.empty(N_NODES, dtype=np.int64)
    for node in order:
        ai, bi = da[node], db[node]
        feas = (n_left > 0) & (a_left >= ai) & (b_left >= bi)
        if not feas.any():
            return None
        # prefer the bin whose remaining a:b slack best matches this node's
        # skew, with a tie-break toward emptier bins
        slack = np.minimum(a_left - ai, b_left - bi).astype(np.float64)
        score = slack + 0.25 * (a_left + b_left - ai - bi)
        score[~feas] = -1e18
        bsel = int(np.argmax(score))
        assign[node] = bsel
        a_left[bsel] -= ai
        b_left[bsel] -= bi
        n_left[bsel] -= 1
    bins = [np.where(assign == b)[0] for b in range(NBINS)]
    return bins


def _make_plan(src, dst):
    src = np.asarray(src, dtype=np.int64)
    dst = np.asarray(dst, dtype=np.int64)
    isa = src < VHALF
    da = np.bincount(dst[isa], minlength=N_NODES)
    db = np.bincount(dst[~isa], minlength=N_NODES)

    for extra14 in (0, 2, 4, 8, 16, 25):
        caps = _slot_caps(extra14)
        bins = _pack_nodes(da, db, caps)
        if bins is not None:
            break
    else:
        raise RuntimeError("node packing failed")

    KA = np.array([c[0] for c in caps], dtype=np.int64)
    KB = np.array([c[1] for c in caps], dtype=np.int64)
    TA = int(KA.sum())
    TB = int(KB.sum())
    T_tot = TA + TB
    a_base = np.concatenate([[0], np.cumsum(KA)])
    b_base = np.concatenate([[0], np.cumsum(KB)])

    # per-edge bin and lane-in-bin
    node_bin = np.empty(N_NODES, dtype=np.int64)
    node_lane = np.empty(N_NODES, dtype=np.int64)
    for bid, nodes in enumerate(bins):
        node_bin[nodes] = bid
        node_lane[nodes] = np.arange(len(nodes))

    ebin = node_bin[dst]
    # order edges by (bin, half, src) for locality and contiguous fill
    order = np.lexsort((src, ~isa, ebin))
    e_bin = ebin[order]
    e_half = (~isa)[order]            # 0 = A, 1 = B
    e_src = src[order]
    e_lane = node_lane[dst[order]]

    # batches of BATCH_SLOTS slots; per batch, A-calls and B-calls chunk the
    # batch's contiguous A/B tile regions into <=CALL_TILES-tile calls.
    # dstloc columns are stored in BATCH order (batch's A tiles then B tiles)
    # so the one-hot is built with ONE tensor_tensor per batch.
    batches = []
    s = 0
    t_base = 0
    dstcol_a = np.zeros(W_SLOTS, dtype=np.int64)   # dstloc col of slot's tile 0 (A)
    dstcol_b = np.zeros(W_SLOTS, dtype=np.int64)
    while s < W_SLOTS:
        s1 = min(s + BATCH_SLOTS, W_SLOTS)
        slots = list(range(s, s1))
        ka_b = int(KA[s:s1].sum())
        kb_b = int(KB[s:s1].sum())
        wins = []
        pa, pb = 0, 0
        for w in slots:
            dstcol_a[w] = t_base + pa
            dstcol_b[w] = t_base + ka_b + pb
            cols = list(range(pa, pa + int(KA[w]))) + \
                   list(range(ka_b + pb, ka_b + pb + int(KB[w])))
            wins.append((w, cols))
            pa += int(KA[w])
            pb += int(KB[w])
        batches.append(dict(ka=ka_b, kb=kb_b, t_base=t_base,
                            a0=int(a_base[s]), b0=int(b_base[s]), wins=wins))
        t_base += ka_b + kb_b
        s = s1

    dstloc = np.full((NC, P, T_tot), -1.0, dtype=np.float64)
    idxA = np.zeros((NC, TA * P), dtype=np.int16)
    idxB = np.zeros((NC, TB * P), dtype=np.int16)

    # bincount per (bin, half)
    key = e_bin * 2 + e_half
    cnt = np.bincount(key, minlength=NBINS * 2)
    starts = np.concatenate([[0], np.cumsum(cnt)])
    for bid in range(NBINS):
        c, s = divmod(bid, W_SLOTS)
        for half in (0, 1):
            k = bid * 2 + half
            e0, e1 = starts[k], starts[k + 1]
            m = e1 - e0
            if m == 0:
                continue
            j = np.arange(m)
            if half == 0:
                assert m <= KA[s] * P, (bid, m, KA[s] * P)
                dstloc[c, j % P, dstcol_a[s] + j // P] = e_lane[e0:e1]
                idxA[c, a_base[s] * P + j] = e_src[e0:e1].astype(np.int16)
            else:
                assert m <= KB[s] * P, (bid, m, KB[s] * P)
                dstloc[c, j % P, dstcol_b[s] + j // P] = e_lane[e0:e1]
                idxB[c, b_base[s] * P + j] = (e_src[e0:e1] - VHALF).astype(np.int16)

    return dict(bins=bins, caps=caps, KA=KA, KB=KB, TA=TA, TB=TB,
                T_tot=T_tot, batches=batches, dstloc=dstloc,
                idxA=idxA, idxB=idxB)


def _wrap16(idx_flat):
    n = idx_flat.shape[0]
    arr = np.empty((16, n // 16), dtype=np.int16)
    j = np.arange(n)
    arr[j % 16, j // 16] = idx_flat
    return np.tile(arr, (8, 1))


def _build_nc(plan):
    TA, TB, T_tot = plan["TA"], plan["TB"], plan["T_tot"]
    # const fp32 column layout
    c_ia = 0                                # idxA int16 [P, TA*8]
    c_ib = c_ia + TA * 4                    # idxB int16 [P, TB*8]
    c_wt = c_ib + TB * 4                    # W^T bf16 [P, P]
    c_brow = c_wt + P // 2                  # bias row bf16 [1, P]
    c_tot = c_brow + P // 2
    plan["c_layout"] = (c_ia, c_ib, c_wt, c_brow, c_tot)

    nc = bacc.Bacc("TRN2", num_swdge_queues=NQ)
    featA = nc.declare_dram_parameter("featA", [VHALF, P], BF16, isOutput=False)
    featB = nc.declare_dram_parameter("featB", [N_NODES - VHALF, P], BF16,
                                      isOutput=False)
    oneh_d = nc.declare_dram_parameter("oneh", [P, T_tot, P], BF16,
                                       isOutput=False)
    consts = nc.declare_dram_parameter("consts", [P, c_tot], F32, isOutput=False)
    out = nc.declare_dram_parameter("out", [W_SLOTS * P, P], F32, isOutput=True)

    with tile.TileContext(nc) as tc:
        with (
            tc.tile_pool(name="const", bufs=1) as const_pool,
            tc.tile_pool(name="msgs", bufs=2) as msgs_pool,
            tc.tile_pool(name="oneh", bufs=2) as oneh_pool,
            tc.tile_pool(name="outp", bufs=3) as out_pool,
            tc.tile_pool(name="psum", bufs=4, space="PSUM") as psum_pool,
        ):
            # split the consts load so the first gathers only wait on the
            # index region, not the whole tile
            cs = const_pool.tile([P, c_wt], F32, tag="cs_idx")
            nc.sync.dma_start(out=cs[:], in_=consts[:, 0:c_wt])
            csm = const_pool.tile([P, c_tot - c_wt], F32, tag="cs_misc")
            nc.sync.dma_start(out=csm[:], in_=consts[:, c_wt:c_tot])
            idxA_sb = cs[:, c_ia:c_ib].bitcast(I16)
            idxB_sb = cs[:, c_ib:c_wt].bitcast(I16)
            wt_sb = csm[:, 0:c_brow - c_wt].bitcast(BF16)
            brow_sb = csm[0:1, c_brow - c_wt:].bitcast(BF16)
            ones_sb = const_pool.tile([1, P], BF16)
            nc.vector.memset(ones_sb[:], 1.0)

            gq = [0]

            def gather_calls(msgs, coff, k_tiles, feat, idx_sb, flat0):
                # chunk [0, k_tiles) into CALL_TILES-tile calls (full tiles
                # first; ring/packet capacity caps a call at 1024 indices)
                sizes = [min(CALL_TILES, k_tiles - o)
                         for o in range(0, k_tiles, CALL_TILES)]
                off = 0
                for nk in sizes:
                    nc.gpsimd.dma_gather(
                        out_ap=msgs[:, coff + off:coff + off + nk, :],
                        in_ap=feat[:],
                        idxs_ap=idx_sb[:, (flat0 + off) * 8:(flat0 + off + nk) * 8],
                        num_idxs=nk * P,
                        num_idxs_reg=nk * P,
                        elem_size=P,
                        queue_num=gq[0] % NQ,
                    )
                    gq[0] += 1
                    off += nk

            for bt in plan["batches"]:
                ka_b, kb_b = bt["ka"], bt["kb"]
                k_b = ka_b + kb_b
                msgs = msgs_pool.tile([P, k_b, P], BF16, tag="msgs")
                gather_calls(msgs, 0, ka_b, featA, idxA_sb, bt["a0"])
                gather_calls(msgs, ka_b, kb_b, featB, idxB_sb, bt["b0"])

                onehot = oneh_pool.tile([P, k_b, P], BF16, tag="onehot")
                # host-precomputed one-hot, streamed via HWDGE (keeps DVE off
                # the SBUF port GpSimd's descriptor rings live on)
                nc.sync.dma_start(
                    out=onehot[:],
                    in_=oneh_d[:, bt["t_base"]:bt["t_base"] + k_b, :])
                for w, cols in bt["wins"]:
                    aggT_ps = psum_pool.tile([P, P], F32, tag="aggT")
                    for i, ccol in enumerate(cols):
                        nc.tensor.matmul(
                            out=aggT_ps[:],
                            lhsT=msgs[:, ccol, :],
                            rhs=onehot[:, ccol, :],
                            start=(i == 0),
                            stop=(i == len(cols) - 1),
                        )
                    aggT_sb = out_pool.tile([P, P], BF16, tag="aggT_sb")
                    nc.scalar.activation(out=aggT_sb[:], in_=aggT_ps[:],
                                         func=mybir.ActivationFunctionType.Copy)
                    out2_ps = psum_pool.tile([P, P], F32, tag="out2")
                    nc.tensor.matmul(out=out2_ps[:], lhsT=aggT_sb[:], rhs=wt_sb,
                                     start=True, stop=False)
                    nc.tensor.matmul(out=out2_ps[:], lhsT=ones_sb[:], rhs=brow_sb,
                                     start=False, stop=True)
                    out_sb = out_pool.tile([P, P], F32, tag="out_sb")
                    nc.scalar.activation(out=out_sb[:], in_=out2_ps[:],
                                         func=mybir.ActivationFunctionType.Relu)
                    nc.sync.dma_start(out=out[w * P:(w + 1) * P, :], in_=out_sb[:])
    nc.finalize()
    return nc


_CACHE = {}


def _prepare(feature, src, dst, W, b):
    feature = np.asarray(feature, dtype=np.float32)
    W = np.asarray(W, dtype=np.float32)
    b = np.asarray(b, dtype=np.float32)
    key = (hash(np.asarray(src).tobytes()), hash(np.asarray(dst).tobytes()))
    if key not in _CACHE:
        plan = _make_plan(src, dst)
        nc = _build_nc(plan)
        _CACHE.clear()
        _CACHE[key] = (plan, nc)
    plan, nc = _CACHE[key]
    c_ia, c_ib, c_wt, c_brow, c_tot = plan["c_layout"]
    TA, TB, T_tot = plan["TA"], plan["TB"], plan["T_tot"]
    featA = np.ascontiguousarray(feature[:VHALF].astype(BF))
    featB = np.ascontiguousarray(feature[VHALF:].astype(BF))
    lanes = np.arange(P, dtype=np.float64)
    in_maps = []
    for c in range(NC):
        consts = np.zeros((P, c_tot), dtype=np.float32)

        def put_bf16(col0, arr2d):
            # arr2d [rows, cols] bf16 -> fp32 columns starting at col0
            a = np.asarray(arr2d, dtype=BF)
            rows, cols = a.shape
            pad = (-cols) % 2
            if pad:
                a = np.concatenate([a, np.zeros((rows, pad), BF)], axis=1)
            a = np.ascontiguousarray(a)
            consts[:rows, col0:col0 + a.shape[1] // 2] = a.view(np.float32)

        consts[:, c_ia:c_ib] = _wrap16(plan["idxA"][c]).view(np.float32)
        consts[:, c_ib:c_wt] = _wrap16(plan["idxB"][c]).view(np.float32)
        put_bf16(c_wt, W.T.astype(BF))
        put_bf16(c_brow, b[None, :].astype(BF))
        oneh = (plan["dstloc"][c][:, :, None] == lanes[None, None, :]).astype(BF)
        in_maps.append({"featA": featA, "featB": featB, "consts": consts,
                        "oneh": np.ascontiguousarray(oneh)})
    return plan, nc, in_maps


def _assemble(plan, results):
    out_full = np.zeros((N_NODES, P), dtype=np.float32)
    for c in range(NC):
        oc = results[c]["out"]
        for s in range(W_SLOTS):
            nodes = plan["bins"][c * W_SLOTS + s]
            if len(nodes):
                out_full[nodes] = oc[s * P:s * P + len(nodes)]
    return out_full


def kernel(feature, src, dst, W, b):
    plan, nc, in_maps = _prepare(feature, src, dst, W, b)
    res = run_bass_kernel_spmd(nc, in_maps, list(range(NC)))
    return _assemble(plan, res.results)


# revision 16
# speedup vs baseline: 1.4408x; 1.0268x over previous
"""GNN message passing (copy_src + segment_sum + Linear + ReLU) on 8 TRN2 cores.

v2. Structure: dst nodes are bin-packed (host side) into 392 windows =
8 cores x 49 slots, <=128 nodes per window, with per-slot uniform edge-tile
capacities (KA_s, KB_s) shared by all cores so the instruction stream is
SPMD-identical. Each core gathers the src rows of its edges from a bf16
replica of the feature table (split in two halves at node 25000 for the
int16 gather-index range) with 1024-index dma_gather calls rotated over
4 SWDGE queues (descriptor generation parallelizes ~4x across queues;
measured 2.6 ns/idx vs 4.6 at 2 queues). Aggregation on-chip: per 128-edge
tile a one-hot(dst_lane) bf16 matrix is built with DVE is_equal, and PE
matmuls accumulate aggT[f, lane] += msgs[e, f].T @ onehot[e, lane] in PSUM
(fp32). Node update per window: aggT -> bf16, matmul with W^T + K=1 bias
matmul, ACT ReLU, DMA out fp32. Host assembles the full output from the
window node lists. No collectives.

Self-contained: shapes hardcoded for feature[50000,128], src/dst[640000],
W[128,128], b[128].
"""
import dataclasses

import numpy as np
import ml_dtypes

import concourse.bacc as bacc
import concourse.tile as tile
from concourse import mybir
from concourse.bass_utils import run_bass_kernel_spmd

P = 128
N_NODES = 50000
N_EDGES = 640000
VHALF = 25000
NC = 8
W_SLOTS = 49
NBINS = NC * W_SLOTS                 # 392 windows
CALL_TILES = 8                       # 1024 idx per dma_gather (ring capacity)
NQ = 4                               # SWDGE queues
BATCH_SLOTS = 8

F32 = mybir.dt.float32
BF16 = mybir.dt.bfloat16
I16 = mybir.dt.int16
BF = ml_dtypes.bfloat16


def _slot_caps(extra14):
    """Per-slot (KA, KB): slots 0..24 are (7,6), 25..48 are (6,7); the first
    `extra14` slots of each group get +1 on the minor side (K=14)."""
    caps = []
    for s in range(W_SLOTS):
        if s < 25:
            ka, kb = 7, 6
            if s < extra14:
                kb += 1
        else:
            ka, kb = 6, 7
            if s - 25 < extra14:
                ka += 1
        caps.append((ka, kb))
    return caps


def _pack_nodes(da, db, caps):
    """Assign all nodes to NBINS bins with per-bin caps (128 nodes,
    KA*128 A-edges, KB*128 B-edges). Vectorized greedy, high-degree first.
    Returns bins: list of node-id arrays, or None if packing fails."""
    n = da + db
    order = np.argsort(-(n.astype(np.int64) * 4 + (da > db)))
    a_cap = np.array([caps[b % W_SLOTS][0] * P for b in range(NBINS)],
                     dtype=np.int64)
    b_cap = np.array([caps[b % W_SLOTS][1] * P for b in range(NBINS)],
                     dtype=np.int64)
    a_left = a_cap.copy()
    b_left = b_cap.copy()
    n_left = np.full(NBINS, P, dtype=np.int64)
    assign = np.empty(N_NODES, dtype=np.int64)
    for node in order:
        ai, bi = da[node], db[node]
        feas = (n_left > 0) & (a_left >= ai) & (b_left >= bi)
        if not feas.any():
            return None
        # prefer the bin whose remaining a:b slack best matches this node's
        # skew, with a tie-break toward emptier bins
        slack = np.minimum(a_left - ai, b_left - bi).astype(np.float64)
        score = slack + 0.25 * (a_left + b_left - ai - bi)
        score[~feas] = -1e18
        bsel = int(np.argmax(score))
        assign[node] = bsel
        a_left[bsel] -= ai
        b_left[bsel] -= bi
        n_left[bsel] -= 1
    bins = [np.where(assign == b)[0] for b in range(NBINS)]
    return bins


def _make_plan(src, dst):
    src = np.asarray(src, dtype=np.int64)
    dst = np.asarray(dst, dtype=np.int64)
    isa = src < VHALF
    da = np.bincount(dst[isa], minlength=N_NODES)
    db = np.bincount(dst[~isa], minlength=N_NODES)

    for extra14 in (0, 2, 4, 8, 16, 25):
        caps = _slot_caps(extra14)
        bins = _pack_nodes(da, db, caps)
        if bins is not None:
            break
    else:
        raise RuntimeError("node packing failed")

    KA = np.array([c[0] for c in caps], dtype=np.int64)
    KB = np.array([c[1] for c in caps], dtype=np.int64)
    TA = int(KA.sum())
    TB = int(KB.sum())
    T_tot = TA + TB
    a_base = np.concatenate([[0], np.cumsum(KA)])
    b_base = np.concatenate([[0], np.cumsum(KB)])

    # per-edge bin and lane-in-bin
    node_bin = np.empty(N_NODES, dtype=np.int64)
    node_lane = np.empty(N_NODES, dtype=np.int64)
    for bid, nodes in enumerate(bins):
        node_bin[nodes] = bid
        node_lane[nodes] = np.arange(len(nodes))

    ebin = node_bin[dst]
    # order edges by (bin, half, src) for locality and contiguous fill
    order = np.lexsort((src, ~isa, ebin))
    e_bin = ebin[order]
    e_half = (~isa)[order]            # 0 = A, 1 = B
    e_src = src[order]
    e_lane = node_lane[dst[order]]

    # batches of BATCH_SLOTS slots; per batch, A-calls and B-calls chunk the
    # batch's contiguous A/B tile regions into <=CALL_TILES-tile calls.
    # dstloc columns are stored in BATCH order (batch's A tiles then B tiles)
    # so the one-hot is built with ONE tensor_tensor per batch.
    batches = []
    s = 0
    t_base = 0
    dstcol_a = np.zeros(W_SLOTS, dtype=np.int64)   # dstloc col of slot's tile 0 (A)
    dstcol_b = np.zeros(W_SLOTS, dtype=np.int64)
    while s < W_SLOTS:
        s1 = min(s + BATCH_SLOTS, W_SLOTS)
        slots = list(range(s, s1))
        ka_b = int(KA[s:s1].sum())
        kb_b = int(KB[s:s1].sum())
        wins = []
        pa, pb = 0, 0
        for w in slots:
            dstcol_a[w] = t_base + pa
            dstcol_b[w] = t_base + ka_b + pb
            cols = list(range(pa, pa + int(KA[w]))) + \
                   list(range(ka_b + pb, ka_b + pb + int(KB[w])))
            wins.append((w, cols))
            pa += int(KA[w])
            pb += int(KB[w])
        batches.append(dict(ka=ka_b, kb=kb_b, t_base=t_base,
                            a0=int(a_base[s]), b0=int(b_base[s]), wins=wins))
        t_base += ka_b + kb_b
        s = s1

    dstloc = np.full((NC, P, T_tot), -1.0, dtype=np.float64)
    idxA = np.zeros((NC, TA * P), dtype=np.int16)
    idxB = np.zeros((NC, TB * P), dtype=np.int16)

    # bincount per (bin, half)
    key = e_bin * 2 + e_half
    cnt = np.bincount(key, minlength=NBINS * 2)
    starts = np.concatenate([[0], np.cumsum(cnt)])
    for bid in range(NBINS):
        c, s = divmod(bid, W_SLOTS)
        for half in (0, 1):
            k = bid * 2 + half
            e0, e1 = starts[k], starts[k + 1]
            m = e1 - e0
            if m == 0:
                continue
            j = np.arange(m)
            if half == 0:
                assert m <= KA[s] * P, (bid, m, KA[s] * P)
                dstloc[c, j % P, dstcol_a[s] + j // P] = e_lane[e0:e1]
                idxA[c, a_base[s] * P + j] = e_src[e0:e1].astype(np.int16)
            else:
                assert m <= KB[s] * P, (bid, m, KB[s] * P)
                dstloc[c, j % P, dstcol_b[s] + j // P] = e_lane[e0:e1]
                idxB[c, b_base[s] * P + j] = (e_src[e0:e1] - VHALF).astype(np.int16)

    return dict(bins=bins, caps=caps, KA=KA, KB=KB, TA=TA, TB=TB,
                T_tot=T_tot, batches=batches, dstloc=dstloc,
                idxA=idxA, idxB=idxB)


def _wrap16(idx_flat):
    n = idx_flat.shape[0]
    arr = np.empty((16, n // 16), dtype=np.int16)
    j = np.arange(n)
    arr[j % 16, j // 16] = idx_flat
    return np.tile(arr, (8, 1))


def _build_nc(plan):
    TA, TB, T_tot = plan["TA"], plan["TB"], plan["T_tot"]
    # const fp32 column layout
    c_ia = 0                                # idxA int16 [P, TA*8]
    c_ib = c_ia + TA * 4                    # idxB int16 [P, TB*8]
    c_wt = c_ib + TB * 4                    # W^T bf16 [P, P]
    c_brow = c_wt + P // 2                  # bias row bf16 [1, P]
    c_tot = c_brow + P // 2
    plan["c_layout"] = (c_ia, c_ib, c_wt, c_brow, c_tot)

    nc = bacc.Bacc("TRN2", num_swdge_queues=NQ)
    featA = nc.declare_dram_parameter("featA", [VHALF, P], BF16, isOutput=False)
    featB = nc.declare_dram_parameter("featB", [N_NODES - VHALF, P], BF16,
                                      isOutput=False)
    oneh_d = nc.declare_dram_parameter("oneh", [P, T_tot, P], BF16,
                                       isOutput=False)
    consts = nc.declare_dram_parameter("consts", [P, c_tot], F32, isOutput=False)
    out = nc.declare_dram_parameter("out", [W_SLOTS * P, P], F32, isOutput=True)

    with tile.TileContext(nc) as tc:
        with (
            tc.tile_pool(name="const", bufs=1) as const_pool,
            tc.tile_pool(name="msgs", bufs=3) as msgs_pool,
            tc.tile_pool(name="oneh", bufs=3) as oneh_pool,
            tc.tile_pool(name="outp", bufs=3) as out_pool,
            tc.tile_pool(name="psum", bufs=4, space="PSUM") as psum_pool,
        ):
            # split the consts load so the first gathers only wait on the
            # index region, not the whole tile
            cs = const_pool.tile([P, c_wt], F32, tag="cs_idx")
            nc.sync.dma_start(out=cs[:], in_=consts[:, 0:c_wt])
            csm = const_pool.tile([P, c_tot - c_wt], F32, tag="cs_misc")
            nc.sync.dma_start(out=csm[:], in_=consts[:, c_wt:c_tot])
            idxA_sb = cs[:, c_ia:c_ib].bitcast(I16)
            idxB_sb = cs[:, c_ib:c_wt].bitcast(I16)
            wt_sb = csm[:, 0:c_brow - c_wt].bitcast(BF16)
            brow_sb = csm[0:1, c_brow - c_wt:].bitcast(BF16)
            ones_sb = const_pool.tile([1, P], BF16)
            nc.vector.memset(ones_sb[:], 1.0)

            gq = [0]

            def gather_calls(msgs, coff, k_tiles, feat, idx_sb, flat0):
                # chunk [0, k_tiles) into CALL_TILES-tile calls (full tiles
                # first; ring/packet capacity caps a call at 1024 indices)
                sizes = [min(CALL_TILES, k_tiles - o)
                         for o in range(0, k_tiles, CALL_TILES)]
                off = 0
                for nk in sizes:
                    nc.gpsimd.dma_gather(
                        out_ap=msgs[:, coff + off:coff + off + nk, :],
                        in_ap=feat[:],
                        idxs_ap=idx_sb[:, (flat0 + off) * 8:(flat0 + off + nk) * 8],
                        num_idxs=nk * P,
                        num_idxs_reg=nk * P,
                        elem_size=P,
                        queue_num=gq[0] % NQ,
                    )
                    gq[0] += 1
                    off += nk

            for bt in plan["batches"]:
                ka_b, kb_b = bt["ka"], bt["kb"]
                k_b = ka_b + kb_b
                msgs = msgs_pool.tile([P, k_b, P], BF16, tag="msgs")
                gather_calls(msgs, 0, ka_b, featA, idxA_sb, bt["a0"])
                gather_calls(msgs, ka_b, kb_b, featB, idxB_sb, bt["b0"])

                onehot = oneh_pool.tile([P, k_b, P], BF16, tag="onehot")
                # host-precomputed one-hot, streamed via HWDGE (keeps DVE off
                # the SBUF port GpSimd's descriptor rings live on)
                nc.sync.dma_start(
                    out=onehot[:],
                    in_=oneh_d[:, bt["t_base"]:bt["t_base"] + k_b, :])
                for w, cols in bt["wins"]:
                    aggT_ps = psum_pool.tile([P, P], F32, tag="aggT")
                    for i, ccol in enumerate(cols):
                        nc.tensor.matmul(
                            out=aggT_ps[:],
                            lhsT=msgs[:, ccol, :],
                            rhs=onehot[:, ccol, :],
                            start=(i == 0),
                            stop=(i == len(cols) - 1),
                        )
                    aggT_sb = out_pool.tile([P, P], BF16, tag="aggT_sb")
                    nc.scalar.activation(out=aggT_sb[:], in_=aggT_ps[:],
                                         func=mybir.ActivationFunctionType.Copy)
                    out2_ps = psum_pool.tile([P, P], F32, tag="out2")
                    nc.tensor.matmul(out=out2_ps[:], lhsT=aggT_sb[:], rhs=wt_sb,
                                     start=True, stop=False)
                    nc.tensor.matmul(out=out2_ps[:], lhsT=ones_sb[:], rhs=brow_sb,
                                     start=False, stop=True)
                    out_sb = out_pool.tile([P, P], F32, tag="out_sb")
                    nc.scalar.activation(out=out_sb[:], in_=out2_ps[:],
                                         func=mybir.ActivationFunctionType.Relu)
                    nc.scalar.dma_start(out=out[w * P:(w + 1) * P, :], in_=out_sb[:])
    nc.finalize()
    return nc


_CACHE = {}


def _prepare(feature, src, dst, W, b):
    feature = np.asarray(feature, dtype=np.float32)
    W = np.asarray(W, dtype=np.float32)
    b = np.asarray(b, dtype=np.float32)
    key = (hash(np.asarray(src).tobytes()), hash(np.asarray(dst).tobytes()))
    if key not in _CACHE:
        plan = _make_plan(src, dst)
        nc = _build_nc(plan)
        _CACHE.clear()
        _CACHE[key] = (plan, nc)
    plan, nc = _CACHE[key]
    c_ia, c_ib, c_wt, c_brow, c_tot = plan["c_layout"]
    TA, TB, T_tot = plan["TA"], plan["TB"], plan["T_tot"]
    featA = np.ascontiguousarray(feature[:VHALF].astype(BF))
    featB = np.ascontiguousarray(feature[VHALF:].astype(BF))
    lanes = np.arange(P, dtype=np.float64)
    in_maps = []
    for c in range(NC):
        consts = np.zeros((P, c_tot), dtype=np.float32)

        def put_bf16(col0, arr2d):
            # arr2d [rows, cols] bf16 -> fp32 columns starting at col0
            a = np.asarray(arr2d, dtype=BF)
            rows, cols = a.shape
            pad = (-cols) % 2
            if pad:
                a = np.concatenate([a, np.zeros((rows, pad), BF)], axis=1)
            a = np.ascontiguousarray(a)
            consts[:rows, col0:col0 + a.shape[1] // 2] = a.view(np.float32)

        consts[:, c_ia:c_ib] = _wrap16(plan["idxA"][c]).view(np.float32)
        consts[:, c_ib:c_wt] = _wrap16(plan["idxB"][c]).view(np.float32)
        put_bf16(c_wt, W.T.astype(BF))
        put_bf16(c_brow, b[None, :].astype(BF))
        oneh = (plan["dstloc"][c][:, :, None] == lanes[None, None, :]).astype(BF)
        in_maps.append({"featA": featA, "featB": featB, "consts": consts,
                        "oneh": np.ascontiguousarray(oneh)})
    return plan, nc, in_maps


def _assemble(plan, results):
    out_full = np.zeros((N_NODES, P), dtype=np.float32)
    for c in range(NC):
        oc = results[c]["out"]
        for s in range(W_SLOTS):
            nodes = plan["bins"][c * W_SLOTS + s]
            if len(nodes):
                out_full[nodes] = oc[s * P:s * P + len(nodes)]
    return out_full


def kernel(feature, src, dst, W, b):
    plan, nc, in_maps = _prepare(feature, src, dst, W, b)
    res = run_bass_kernel_spmd(nc, in_maps, list(range(NC)))
    return _assemble(plan, res.results)


# revision 17
# speedup vs baseline: 1.4530x; 1.0084x over previous
"""GNN message passing (copy_src + segment_sum + Linear + ReLU) on 8 TRN2 cores.

Structure: dst nodes are bin-packed (host side) into 392 windows = 8 cores
x 49 slots, <=128 nodes per window, with per-slot uniform edge-tile
capacities (KA_s, KB_s) shared by all cores so the instruction stream is
SPMD-identical. Each core gathers the src rows of its edges from a bf16
replica of the feature table (split in two halves at node 25000 for the
int16 gather-index range) with 1024-index single-packet dma_gather calls
rotated over 4 SWDGE queues — descriptor generation is the bottleneck and
parallelizes ~4x across queues (measured 2.2-2.6 ns/idx vs 4.6 at 2 queues;
bigger calls, multi-packet mode, or a bigger descriptor carveout are all
2.5x slower or wedge the device). The per-edge one-hot(dst lane) matrices
are precomputed on the host and streamed from HBM via HWDGE — building them
with a DVE tensor_tensor stalls SWDGE descriptor generation (shared SBUF
port with the Q7 descriptor rings). PE matmuls accumulate
aggT[f, lane] += msgs[e, f].T @ onehot[e, lane] in PSUM (fp32). Node update
per window: ACT copies aggT to bf16, matmul with W^T + K=1 bias matmul,
ACT ReLU, DMA out fp32 on the ACT HWDGE ring. Host assembles the full
output from the window node lists. No collectives.

Self-contained: shapes hardcoded for feature[50000,128], src/dst[640000],
W[128,128], b[128].
"""
import numpy as np
import ml_dtypes

import concourse.bacc as bacc
import concourse.tile as tile
from concourse import mybir
from concourse.bass_utils import run_bass_kernel_spmd

P = 128
N_NODES = 50000
N_EDGES = 640000
VHALF = 25000
NC = 8
W_SLOTS = 49
NBINS = NC * W_SLOTS                 # 392 windows
CALL_TILES = 8                       # 1024 idx per dma_gather (ring capacity)
NQ = 4                               # SWDGE queues
BATCH_SLOTS = 8

F32 = mybir.dt.float32
BF16 = mybir.dt.bfloat16
I16 = mybir.dt.int16
BF = ml_dtypes.bfloat16


def _slot_caps(extra14):
    """Per-slot (KA, KB): slots 0..24 are (7,6), 25..48 are (6,7); the first
    `extra14` slots of each group get +1 on the minor side (K=14)."""
    caps = []
    for s in range(W_SLOTS):
        if s < 25:
            ka, kb = 7, 6
            if s < extra14:
                kb += 1
        else:
            ka, kb = 6, 7
            if s - 25 < extra14:
                ka += 1
        caps.append((ka, kb))
    return caps


def _pack_nodes(da, db, caps):
    """Assign all nodes to NBINS bins with per-bin caps (128 nodes,
    KA*128 A-edges, KB*128 B-edges). Vectorized greedy, high-degree first.
    Returns bins: list of node-id arrays, or None if packing fails."""
    n = da + db
    order = np.argsort(-(n.astype(np.int64) * 4 + (da > db)))
    a_cap = np.array([caps[b % W_SLOTS][0] * P for b in range(NBINS)],
                     dtype=np.int64)
    b_cap = np.array([caps[b % W_SLOTS][1] * P for b in range(NBINS)],
                     dtype=np.int64)
    a_left = a_cap.copy()
    b_left = b_cap.copy()
    n_left = np.full(NBINS, P, dtype=np.int64)
    assign = np.empty(N_NODES, dtype=np.int64)
    for node in order:
        ai, bi = da[node], db[node]
        feas = (n_left > 0) & (a_left >= ai) & (b_left >= bi)
        if not feas.any():
            return None
        # prefer the bin whose remaining a:b slack best matches this node's
        # skew, with a tie-break toward emptier bins
        slack = np.minimum(a_left - ai, b_left - bi).astype(np.float64)
        score = slack + 0.25 * (a_left + b_left - ai - bi)
        score[~feas] = -1e18
        bsel = int(np.argmax(score))
        assign[node] = bsel
        a_left[bsel] -= ai
        b_left[bsel] -= bi
        n_left[bsel] -= 1
    bins = [np.where(assign == b)[0] for b in range(NBINS)]
    return bins


def _make_plan(src, dst):
    src = np.asarray(src, dtype=np.int64)
    dst = np.asarray(dst, dtype=np.int64)
    isa = src < VHALF
    da = np.bincount(dst[isa], minlength=N_NODES)
    db = np.bincount(dst[~isa], minlength=N_NODES)

    for extra14 in (0, 2, 4, 8, 16, 25):
        caps = _slot_caps(extra14)
        bins = _pack_nodes(da, db, caps)
        if bins is not None:
            break
    else:
        raise RuntimeError("node packing failed")

    KA = np.array([c[0] for c in caps], dtype=np.int64)
    KB = np.array([c[1] for c in caps], dtype=np.int64)
    TA = int(KA.sum())
    TB = int(KB.sum())
    T_tot = TA + TB
    a_base = np.concatenate([[0], np.cumsum(KA)])
    b_base = np.concatenate([[0], np.cumsum(KB)])

    # per-edge bin and lane-in-bin
    node_bin = np.empty(N_NODES, dtype=np.int64)
    node_lane = np.empty(N_NODES, dtype=np.int64)
    for bid, nodes in enumerate(bins):
        node_bin[nodes] = bid
        node_lane[nodes] = np.arange(len(nodes))

    ebin = node_bin[dst]
    # order edges by (bin, half, src) for locality and contiguous fill
    order = np.lexsort((src, ~isa, ebin))
    e_bin = ebin[order]
    e_half = (~isa)[order]            # 0 = A, 1 = B
    e_src = src[order]
    e_lane = node_lane[dst[order]]

    # batches of BATCH_SLOTS slots; per batch, A-calls and B-calls chunk the
    # batch's contiguous A/B tile regions into <=CALL_TILES-tile calls.
    # dstloc columns are stored in BATCH order (batch's A tiles then B tiles)
    # so the one-hot is built with ONE tensor_tensor per batch.
    batches = []
    s = 0
    t_base = 0
    dstcol_a = np.zeros(W_SLOTS, dtype=np.int64)   # dstloc col of slot's tile 0 (A)
    dstcol_b = np.zeros(W_SLOTS, dtype=np.int64)
    while s < W_SLOTS:
        s1 = min(s + BATCH_SLOTS, W_SLOTS)
        slots = list(range(s, s1))
        ka_b = int(KA[s:s1].sum())
        kb_b = int(KB[s:s1].sum())
        wins = []
        pa, pb = 0, 0
        for w in slots:
            dstcol_a[w] = t_base + pa
            dstcol_b[w] = t_base + ka_b + pb
            cols = list(range(pa, pa + int(KA[w]))) + \
                   list(range(ka_b + pb, ka_b + pb + int(KB[w])))
            wins.append((w, cols))
            pa += int(KA[w])
            pb += int(KB[w])
        batches.append(dict(ka=ka_b, kb=kb_b, t_base=t_base,
                            a0=int(a_base[s]), b0=int(b_base[s]), wins=wins))
        t_base += ka_b + kb_b
        s = s1

    dstloc = np.full((NC, P, T_tot), -1.0, dtype=np.float64)
    idxA = np.zeros((NC, TA * P), dtype=np.int16)
    idxB = np.zeros((NC, TB * P), dtype=np.int16)

    # bincount per (bin, half)
    key = e_bin * 2 + e_half
    cnt = np.bincount(key, minlength=NBINS * 2)
    starts = np.concatenate([[0], np.cumsum(cnt)])
    for bid in range(NBINS):
        c, s = divmod(bid, W_SLOTS)
        for half in (0, 1):
            k = bid * 2 + half
            e0, e1 = starts[k], starts[k + 1]
            m = e1 - e0
            if m == 0:
                continue
            j = np.arange(m)
            if half == 0:
                assert m <= KA[s] * P, (bid, m, KA[s] * P)
                dstloc[c, j % P, dstcol_a[s] + j // P] = e_lane[e0:e1]
                idxA[c, a_base[s] * P + j] = e_src[e0:e1].astype(np.int16)
            else:
                assert m <= KB[s] * P, (bid, m, KB[s] * P)
                dstloc[c, j % P, dstcol_b[s] + j // P] = e_lane[e0:e1]
                idxB[c, b_base[s] * P + j] = (e_src[e0:e1] - VHALF).astype(np.int16)

    return dict(bins=bins, caps=caps, KA=KA, KB=KB, TA=TA, TB=TB,
                T_tot=T_tot, batches=batches, dstloc=dstloc,
                idxA=idxA, idxB=idxB)


def _wrap16(idx_flat):
    n = idx_flat.shape[0]
    arr = np.empty((16, n // 16), dtype=np.int16)
    j = np.arange(n)
    arr[j % 16, j // 16] = idx_flat
    return np.tile(arr, (8, 1))


def _build_nc(plan):
    TA, TB, T_tot = plan["TA"], plan["TB"], plan["T_tot"]
    # const fp32 column layout
    c_ia = 0                                # idxA int16 [P, TA*8]
    c_ib = c_ia + TA * 4                    # idxB int16 [P, TB*8]
    c_wt = c_ib + TB * 4                    # W^T bf16 [P, P]
    c_brow = c_wt + P // 2                  # bias row bf16 [1, P]
    c_tot = c_brow + P // 2
    plan["c_layout"] = (c_ia, c_ib, c_wt, c_brow, c_tot)

    nc = bacc.Bacc("TRN2", num_swdge_queues=NQ)
    featA = nc.declare_dram_parameter("featA", [VHALF, P], BF16, isOutput=False)
    featB = nc.declare_dram_parameter("featB", [N_NODES - VHALF, P], BF16,
                                      isOutput=False)
    oneh_d = nc.declare_dram_parameter("oneh", [P, T_tot, P], BF16,
                                       isOutput=False)
    consts = nc.declare_dram_parameter("consts", [P, c_tot], F32, isOutput=False)
    out = nc.declare_dram_parameter("out", [W_SLOTS * P, P], F32, isOutput=True)

    with tile.TileContext(nc) as tc:
        with (
            tc.tile_pool(name="const", bufs=1) as const_pool,
            tc.tile_pool(name="msgs", bufs=3) as msgs_pool,
            tc.tile_pool(name="oneh", bufs=3) as oneh_pool,
            tc.tile_pool(name="outp", bufs=3) as out_pool,
            tc.tile_pool(name="psum", bufs=4, space="PSUM") as psum_pool,
        ):
            # split the consts load so the first gathers only wait on the
            # index region, not the whole tile
            cs = const_pool.tile([P, c_wt], F32, tag="cs_idx")
            nc.sync.dma_start(out=cs[:], in_=consts[:, 0:c_wt])
            csm = const_pool.tile([P, c_tot - c_wt], F32, tag="cs_misc")
            nc.sync.dma_start(out=csm[:], in_=consts[:, c_wt:c_tot])
            idxA_sb = cs[:, c_ia:c_ib].bitcast(I16)
            idxB_sb = cs[:, c_ib:c_wt].bitcast(I16)
            wt_sb = csm[:, 0:c_brow - c_wt].bitcast(BF16)
            brow_sb = csm[0:1, c_brow - c_wt:].bitcast(BF16)
            ones_sb = const_pool.tile([1, P], BF16)
            nc.vector.memset(ones_sb[:], 1.0)

            gq = [0]

            def gather_calls(msgs, coff, k_tiles, feat, idx_sb, flat0):
                # chunk [0, k_tiles) into CALL_TILES-tile calls (full tiles
                # first; ring/packet capacity caps a call at 1024 indices)
                sizes = [min(CALL_TILES, k_tiles - o)
                         for o in range(0, k_tiles, CALL_TILES)]
                off = 0
                for nk in sizes:
                    nc.gpsimd.dma_gather(
                        out_ap=msgs[:, coff + off:coff + off + nk, :],
                        in_ap=feat[:],
                        idxs_ap=idx_sb[:, (flat0 + off) * 8:(flat0 + off + nk) * 8],
                        num_idxs=nk * P,
                        num_idxs_reg=nk * P,
                        elem_size=P,
                        queue_num=gq[0] % NQ,
                    )
                    gq[0] += 1
                    off += nk

            for bt in plan["batches"]:
                ka_b, kb_b = bt["ka"], bt["kb"]
                k_b = ka_b + kb_b
                msgs = msgs_pool.tile([P, k_b, P], BF16, tag="msgs")
                gather_calls(msgs, 0, ka_b, featA, idxA_sb, bt["a0"])
                gather_calls(msgs, ka_b, kb_b, featB, idxB_sb, bt["b0"])

                onehot = oneh_pool.tile([P, k_b, P], BF16, tag="onehot")
                # host-precomputed one-hot, streamed via HWDGE (keeps DVE off
                # the SBUF port GpSimd's descriptor rings live on)
                nc.sync.dma_start(
                    out=onehot[:],
                    in_=oneh_d[:, bt["t_base"]:bt["t_base"] + k_b, :])
                for w, cols in bt["wins"]:
                    aggT_ps = psum_pool.tile([P, P], F32, tag="aggT")
                    for i, ccol in enumerate(cols):
                        nc.tensor.matmul(
                            out=aggT_ps[:],
                            lhsT=msgs[:, ccol, :],
                            rhs=onehot[:, ccol, :],
                            start=(i == 0),
                            stop=(i == len(cols) - 1),
                        )
                    aggT_sb = out_pool.tile([P, P], BF16, tag="aggT_sb")
                    nc.scalar.activation(out=aggT_sb[:], in_=aggT_ps[:],
                                         func=mybir.ActivationFunctionType.Copy)
                    out2_ps = psum_pool.tile([P, P], F32, tag="out2")
                    nc.tensor.matmul(out=out2_ps[:], lhsT=aggT_sb[:], rhs=wt_sb,
                                     start=True, stop=False)
                    nc.tensor.matmul(out=out2_ps[:], lhsT=ones_sb[:], rhs=brow_sb,
                                     start=False, stop=True)
                    out_sb = out_pool.tile([P, P], F32, tag="out_sb")
                    nc.scalar.activation(out=out_sb[:], in_=out2_ps[:],
                                         func=mybir.ActivationFunctionType.Relu)
                    nc.scalar.dma_start(out=out[w * P:(w + 1) * P, :], in_=out_sb[:])
    nc.finalize()
    return nc


_CACHE = {}


def _prepare(feature, src, dst, W, b):
    feature = np.asarray(feature, dtype=np.float32)
    W = np.asarray(W, dtype=np.float32)
    b = np.asarray(b, dtype=np.float32)
    key = (hash(np.asarray(src).tobytes()), hash(np.asarray(dst).tobytes()))
    if key not in _CACHE:
        plan = _make_plan(src, dst)
        nc = _build_nc(plan)
        _CACHE.clear()
        _CACHE[key] = (plan, nc)
    plan, nc = _CACHE[key]
    c_ia, c_ib, c_wt, c_brow, c_tot = plan["c_layout"]
    TA, TB, T_tot = plan["TA"], plan["TB"], plan["T_tot"]
    featA = np.ascontiguousarray(feature[:VHALF].astype(BF))
    featB = np.ascontiguousarray(feature[VHALF:].astype(BF))
    lanes = np.arange(P, dtype=np.float64)
    in_maps = []
    for c in range(NC):
        consts = np.zeros((P, c_tot), dtype=np.float32)

        def put_bf16(col0, arr2d):
            # arr2d [rows, cols] bf16 -> fp32 columns starting at col0
            a = np.asarray(arr2d, dtype=BF)
            rows, cols = a.shape
            pad = (-cols) % 2
            if pad:
                a = np.concatenate([a, np.zeros((rows, pad), BF)], axis=1)
            a = np.ascontiguousarray(a)
            consts[:rows, col0:col0 + a.shape[1] // 2] = a.view(np.float32)

        consts[:, c_ia:c_ib] = _wrap16(plan["idxA"][c]).view(np.float32)
        consts[:, c_ib:c_wt] = _wrap16(plan["idxB"][c]).view(np.float32)
        put_bf16(c_wt, W.T.astype(BF))
        put_bf16(c_brow, b[None, :].astype(BF))
        oneh = (plan["dstloc"][c][:, :, None] == lanes[None, None, :]).astype(BF)
        in_maps.append({"featA": featA, "featB": featB, "consts": consts,
                        "oneh": np.ascontiguousarray(oneh)})
    return plan, nc, in_maps


def _assemble(plan, results):
    out_full = np.zeros((N_NODES, P), dtype=np.float32)
    for c in range(NC):
        oc = results[c]["out"]
        for s in range(W_SLOTS):
            nodes = plan["bins"][c * W_SLOTS + s]
            if len(nodes):
                out_full[nodes] = oc[s * P:s * P + len(nodes)]
    return out_full


def kernel(feature, src, dst, W, b):
    plan, nc, in_maps = _prepare(feature, src, dst, W, b)
    res = run_bass_kernel_spmd(nc, in_maps, list(range(NC)))
    return _assemble(plan, res.results)
